# revision 64
# baseline (speedup 1.0000x reference)
"""BitLinear (1-bit packed weights) on 8 TRN2 NeuronCores.

out = x @ W.T, x [64, 4096] f32, W [11008, 4096] in {-1,+1} unpacked from
bp (one byte per int32, MSB-first bits).

Strategy (tensor-parallel, no collectives):
 - shard out_features 11008 -> 8 x 1376 rows of W; x replicated.
 - host: repack bp bytes into dense 16-bit words, transposed to
   [word-idx, n]: btw [128, 2752] (both 128-word chunks side by side).
 - rank-1 trick: out = sum_k 2*x_k*b_k - rowsum(x) with b in {0,1}, so
   each bit-plane needs only ONE tensor_scalar(AND[,OR]) -> bf16-coded
   u16 (host pre-scales the matching x block by 2^(1-s)); corrections
   land as a per-partition scalar add in the PSUM->SBUF copies.
 - plane production split: DVE 14 planes (4x mode, ~0.79us each; the
   first SPLIT_HEAD planes as chunk-halves to fill the btw-chunk-1 DMA
   gap), ACT 2 planes: bits 7&15 in ONE Sign pass over the uint8 view
   (w = 2b-1).  GPSIMD must NOT run tensor ops concurrently with DVE 4x
   ops (shared SBUF port-pair lock); GPSIMD cannot read PSUM.
 - DMA: btw chunk 0 on the sync HWDGE ring, chunk 1 on the scalar ring
   (the second ring's first byte lags ~2.5us; the DVE chunk-0 half ops
   cover that gap); xb staged behind btw on the sync ring.
 - PE: column-tiled pairs (0,0)/(0,64); asymmetric psum regions
   (432, 256) so the final drain+DMA tail is short; junk-matmul HAM
   warmup + tiny pad matmuls to keep the PE clock un-throttled.
"""

import math
import sys

sys.path.insert(0, "/opt/trn_rl_repo")

import ml_dtypes
import numpy as np

import concourse.bass as bass
import concourse.mybir as mybir
from concourse.bass_utils import run_bass_kernel_spmd

OUT_F = 11008
IN_F = 4096
M = 64
NCORES = 8
NSH = OUT_F // NCORES  # 1376 rows of W per core
NSH2 = 2 * NSH  # free width of btw / u tiles (both chunks)

PACK = 16  # bits per packed word on device
NW = IN_F // PACK  # packed words along k per W row (256)
NCH = NW // 128  # 128-partition word chunks (2)
NPAIR = PACK  # 16 planes (o = bit offset in word)
NA = NSH // 2  # 688 output columns per column-tile half
QSPLITS = (432, 256)  # psum n-chunks per half (small tail region)
HALF = 688  # btw piece width (one chunk half)

_dt_word = mybir.dt.uint16
_np_word = "<u2"

N_WARMUP = 8  # junk PE matmuls (N=512): HAM warmup + bridge to real work
SPLIT_HEAD = 2  # leading planes produced as chunk-halves (DMA gap fill)
PAD_JUNK = True  # tiny junk MM after each plane to keep HAM warm
DEBUG_DUMP = False  # add dram outputs with the produced planes

OMEGA = 2.0 * math.pi / 32768.0  # period 2^15 in v extracts bit 14

# plane consumption order (PE program order). Entries: (engine, o).
# act planes: o=0 (bit7 of low byte), o=8 (bit15); one Sign pass over the
# uint8 view produces both.  (bit14 via Sign(Sin(.)) does NOT work: the
# HW sin spline is garbage beyond ~+-1.8pi, ~11% wrong signs.)
SCHED = [
    ("dve", 2),
    ("dve", 3),
    ("dve", 4),
    ("dve", 5),
    ("act", 0),
    ("act", 8),
    ("dve", 6),
    ("dve", 7),
    ("dve", 10),
    ("dve", 11),
    ("dve", 12),
    ("dve", 13),
    ("dve", 14),
    ("act", 1),
    ("act", 9),
    ("dve", 15),
]

DVE_PLANES = [o for eng, o in SCHED if eng == "dve"]  # production order


def _shift(o):
    # word bit position holding k-offset o (little-endian byte packing,
    # MSB-first bit order inside each byte)
    return 8 * (o // 8) + 7 - (o % 8)


def _bf16_of_bits(bits):
    return float(np.uint16(bits).view(ml_dtypes.bfloat16))


def _recipe(s):
    """Plane-extraction recipe for bit position s (s not in {7,14,15}).

    Returns (mask, orconst, alpha, beta): the DVE op is
      r_u16 = (v & mask) [| orconst]
    and reinterpreting r as bf16 gives  w = alpha + beta * bit.
    """
    if 9 <= s <= 14:
        # single exponent-field bit: {0, 2^(2^(s-7)-127)}, both exact
        return (1 << s, None, 0.0, _bf16_of_bits(1 << s))
    if s == 8:
        # exp bits: 0x3E80=0.25 -> OR 1<<8 -> 0x3F80=1.0
        return (1 << s, 0x3E80, 0.25, 0.75)
    if s == 7:
        # low exp bit: 0x3F00=0.5 -> OR 1<<7 -> 0x3F80=1.0
        return (1 << s, 0x3F00, 0.5, 0.5)
    # mantissa bit: 1.0 -> 1.0 + 2^(s-7)
    return (1 << s, 0x3F80, 1.0, float(2.0 ** (s - 7)))


def _build():
    nc = bass.Bass()
    # dram: btw halves contiguous ([0:128] = chunk 0 words, [128:256] = chunk 1)
    bpt = nc.declare_dram_parameter("bpt", [256, NSH], _dt_word, isOutput=False)
    xr = nc.declare_dram_parameter(
        "xr", [128, 2 * NPAIR * M], mybir.dt.bfloat16, isOutput=False
    )
    rs = nc.declare_dram_parameter("rs", [128, 1], mybir.dt.float32, isOutput=False)
    out = nc.declare_dram_parameter("out", [M, NSH], mybir.dt.float32, isOutput=True)
    dbgu = dbgs = None
    if DEBUG_DUMP:
        dbgu = nc.declare_dram_parameter(
            "dbgu", [128, NPAIR * NSH2], mybir.dt.uint16, isOutput=True
        )
        dbgs = nc.declare_dram_parameter(
            "dbgs", [128, NSH2 * 2], mybir.dt.uint16, isOutput=True
        )

    A = mybir.AluOpType
    AF = mybir.ActivationFunctionType

    slot = {o: pos for pos, (_, o) in enumerate(SCHED)}
    # DVE op order: [head c0 halves..., shift-c0, head c1 halves...,
    # shift-c1, full planes...].  The shift ops sit AFTER the head halves
    # so they fill DMA-wait gaps instead of delaying PE consumption.
    #   head plane i: c0 op = i+1, c1 op = SPLIT_HEAD+2+i
    #   shift ops: SPLIT_HEAD+1 and 2*(SPLIT_HEAD+1)
    #   full plane i >= SPLIT_HEAD: op = i + SPLIT_HEAD + 3
    v_cnt = {
        o: (SPLIT_HEAD + i + 3) for i, o in enumerate(DVE_PLANES)
    }
    a_cnt = {0: 1, 8: 1, 1: 2, 9: 2}

    from contextlib import ExitStack

    with ExitStack() as stack:
        ec = stack.enter_context
        xb = ec(nc.sbuf_tensor("xb", [128, 2 * NPAIR * M], mybir.dt.bfloat16))
        btw = ec(nc.sbuf_tensor("btw", [128, NSH2], _dt_word))
        btw2 = ec(nc.sbuf_tensor("btw2", [128, NSH2], _dt_word))
        u = ec(nc.sbuf_tensor("u", [128, NPAIR, NSH2], mybir.dt.bfloat16))
        # sg2[:, w, j]: j=0 -> plane o=0 (bit7 of low byte of word w),
        # j=1 -> plane o=8 (bit15); written densely by ACT Sign passes.
        # sg2b: same for btw2 = btw << 1, giving o=1 (bit6) / o=9 (bit14).
        sg2 = ec(nc.sbuf_tensor("sg2", [128, NSH2, 2], mybir.dt.bfloat16))
        sg2b = ec(nc.sbuf_tensor("sg2b", [128, NSH2, 2], mybir.dt.bfloat16))
        ot = ec(nc.sbuf_tensor("ot", [128, NA], mybir.dt.float32))
        rsb = ec(nc.sbuf_tensor("rsb", [128, 1], mybir.dt.float32))
        sgb8 = ec(nc.sbuf_tensor("sgb8", [128, 1], mybir.dt.float32))
        junk = ec(nc.sbuf_tensor("junk", [128, 512], mybir.dt.bfloat16))
        scr = ec(nc.sbuf_tensor("scr", [1, 1], mybir.dt.float32))
        q0 = ec(nc.psum_tensor("q0", [128, QSPLITS[0]], mybir.dt.float32))
        q1 = ec(nc.psum_tensor("q1", [128, QSPLITS[1]], mybir.dt.float32))
        psw = ec(nc.psum_tensor("psw", [M, 512], mybir.dt.float32))
        sc = ec(nc.semaphore("sc"))  # gpsimd const memsets
        ss = ec(nc.semaphore("ss"))  # rs DMA
        sx1 = ec(nc.semaphore("sx1"))  # xb cols 0:512 (pos 0-3 blocks)
        sx2 = ec(nc.semaphore("sx2"))  # xb cols 512:1280 (pos 4-9)
        sx3 = ec(nc.semaphore("sx3"))  # xb cols 1280:2048 (pos 10-15)
        sa1 = ec(nc.semaphore("sa1"))  # btw chunk 0 (cols 0:1376)
        sb1 = ec(nc.semaphore("sb1"))  # btw chunk 1 (cols 1376:2752)
        sv = ec(nc.semaphore("sv"))  # DVE plane ops
        sa = ec(nc.semaphore("sa"))  # ACT plane ops
        sp = ec(nc.semaphore("sp"))  # PE psum region completion
        scv = ec(nc.semaphore("scv"))  # DVE psum drains (B half)
        so = ec(nc.semaphore("so"))  # output DMAs
        block = ec(nc.Block())

        qs = [q0, q1]

        def plane_op(eng, o, lo, hi):
            mask, orc, _, _ = _recipe(_shift(o))
            dst = u[:, slot[o], lo:hi].bitcast(_dt_word)
            if orc is None:
                return eng.tensor_scalar(
                    dst, btw[:, lo:hi], mask, None, op0=A.bitwise_and
                )
            return eng.tensor_scalar(
                dst, btw[:, lo:hi], mask, orc,
                op0=A.bitwise_and, op1=A.bitwise_or,
            )

        def moving(pos, o, lo, hi):
            # moving-operand slice for plane at SCHED pos, btw cols [lo:hi)
            if o == 0:
                return sg2[:, lo:hi, 0]
            if o == 8:
                return sg2[:, lo:hi, 1]
            if o == 1:
                return sg2b[:, lo:hi, 0]
            if o == 9:
                return sg2b[:, lo:hi, 1]
            return u[:, pos, lo:hi]

        @block.sync
        def _(sync: bass.BassEngine):
            # btw chunk 0 first; chunk 1 goes on the scalar ring in parallel
            sync.dma_start(out=btw[:, 0:NSH], in_=bpt[0:128, :]).then_inc(sa1, 16)
            sync.dma_start(out=xb[:, 0:512], in_=xr[:, 0:512]).then_inc(sx1, 16)
            sync.dma_start(out=xb[:, 512:1280], in_=xr[:, 512:1280]).then_inc(
                sx2, 16
            )
            sync.dma_start(out=xb[:, 1280:2048], in_=xr[:, 1280:2048]).then_inc(
                sx3, 16
            )
            sync.dma_start(out=rsb[:, :], in_=rs[:, :]).then_inc(ss, 16)
            # output B half (rows 64:128 of psum -> out cols 688:1376)
            sync.wait_ge(scv, 2)
            sync.dma_start(
                out=out[:, NA:NSH], in_=ot[M : 2 * M, :]
            ).then_inc(so, 16)
            if DEBUG_DUMP:
                sync.wait_ge(sv, 17)
                sync.wait_ge(sa, 2)
                sync.dma_start(
                    out=dbgu[:, :],
                    in_=u[:, :, :].bitcast(mybir.dt.uint16),
                ).then_inc(so, 16)
                sync.dma_start(
                    out=dbgs[:, :],
                    in_=sg2[:, :, :].bitcast(mybir.dt.uint16),
                ).then_inc(so, 16)

        @block.scalar
        def _(scalar: bass.BassEngine):
            scalar.dma_start(out=btw[:, NSH:NSH2], in_=bpt[128:256, :]).then_inc(
                sb1, 16
            )
            # warm: trigger the activation table load during the DMA flight
            scalar.activation(scr[:, :], scr[:, :], AF.Sign, bias=scr[:, :], scale=1.0)
            # planes o=0/o=8: Sign over the uint8 view of btw (+-1 exact,
            # w = 2b-1); chunk-split so work starts on btw chunk 0.
            scalar.wait_ge(sc, 1)
            scalar.wait_ge(sa1, 16)
            scalar.activation(
                sg2[:, 0:NSH, :],
                btw[:, 0:NSH].bitcast(mybir.dt.uint8),
                AF.Sign,
                bias=sgb8[:, :],
                scale=1.0,
            )
            scalar.wait_ge(sb1, 16)
            scalar.activation(
                sg2[:, NSH:NSH2, :],
                btw[:, NSH:NSH2].bitcast(mybir.dt.uint8),
                AF.Sign,
                bias=sgb8[:, :],
                scale=1.0,
            ).then_inc(sa)
            # planes o=1/o=9: byte signs of btw2 = btw << 1 (DVE-produced)
            scalar.wait_ge(sv, SPLIT_HEAD + 1)
            scalar.activation(
                sg2b[:, 0:NSH, :],
                btw2[:, 0:NSH].bitcast(mybir.dt.uint8),
                AF.Sign,
                bias=sgb8[:, :],
                scale=1.0,
            )
            scalar.wait_ge(sv, 2 * (SPLIT_HEAD + 1))
            scalar.activation(
                sg2b[:, NSH:NSH2, :],
                btw2[:, NSH:NSH2].bitcast(mybir.dt.uint8),
                AF.Sign,
                bias=sgb8[:, :],
                scale=1.0,
            ).then_inc(sa)
            # drain A half (rows 0:64) + issue its output DMAs
            scalar.wait_ge(ss, 16)
            scalar.wait_ge(sp, 1)
            scalar.activation(
                ot[0:M, 0 : QSPLITS[0]],
                q0[0:M, :],
                AF.Identity,
                bias=rsb[0:M, :],
                scale=1.0,
            )
            scalar.wait_ge(sp, 3)
            scalar.activation(
                ot[0:M, QSPLITS[0] : NA],
                q1[0:M, :],
                AF.Identity,
                bias=rsb[0:M, :],
                scale=1.0,
            )
            scalar.dma_start(out=out[:, 0:NA], in_=ot[0:M, :]).then_inc(so, 16)

        @block.vector
        def _(vector: bass.BassEngine):
            # chunk-0 halves of the head planes while btw chunk 1 flies;
            # the shift copies (feed ACT o=1/o=9 sign passes) fill the
            # remaining DMA-wait time
            vector.wait_ge(sa1, 16)
            for o in DVE_PLANES[:SPLIT_HEAD]:
                plane_op(vector, o, 0, NSH).then_inc(sv)
            vector.tensor_scalar(
                btw2[:, 0:NSH], btw[:, 0:NSH], 1, None,
                op0=A.logical_shift_left,
            ).then_inc(sv)
            vector.wait_ge(sb1, 16)
            for o in DVE_PLANES[:SPLIT_HEAD]:
                plane_op(vector, o, NSH, NSH2).then_inc(sv)
            vector.tensor_scalar(
                btw2[:, NSH:NSH2], btw[:, NSH:NSH2], 1, None,
                op0=A.logical_shift_left,
            ).then_inc(sv)
            for o in DVE_PLANES[SPLIT_HEAD:]:
                plane_op(vector, o, 0, NSH2).then_inc(sv)
            # drain B half (rows 64:128), adding -rowsum correction
            vector.wait_ge(ss, 16)
            vector.wait_ge(sp, 2)
            vector.tensor_scalar(
                ot[M : 2 * M, 0 : QSPLITS[0]],
                q0[M : 2 * M, :],
                rsb[M : 2 * M, :],
                None,
                op0=A.add,
            ).then_inc(scv)
            vector.wait_ge(sp, 4)
            vector.tensor_scalar(
                ot[M : 2 * M, QSPLITS[0] : NA],
                q1[M : 2 * M, :],
                rsb[M : 2 * M, :],
                None,
                op0=A.add,
            ).then_inc(scv)

        @block.gpsimd
        def _(gpsimd: bass.BassEngine):
            gpsimd.memset(sgb8[:, :], -127.5).then_inc(sc)

        @block.tensor
        def _(tensor: bass.BassEngine):
            # HAM warmup on junk data (no DMA dependency)
            for _i in range(N_WARMUP):
                tensor.matmul(
                    psw[:, :], junk[:, 0:M], junk[:, :], start=True, stop=True
                )

            def mm(q, j, lh, src, tile, start, stop):
                i = tensor.matmul(
                    q, lh, src, start=start, stop=stop, tile_position=tile
                )
                i.ins.ldweights = False
                return i

            tensor.wait_ge(sx1, 16)
            for pos in range(NPAIR):
                eng, o = SCHED[pos]
                if pos == 4:
                    tensor.wait_ge(sx2, 16)
                elif pos == 10:
                    tensor.wait_ge(sx3, 16)
                last = pos == NPAIR - 1
                first = pos == 0
                if eng == "dve":
                    head = pos < SPLIT_HEAD
                    if not head:
                        tensor.wait_ge(sv, v_cnt[o])
                else:
                    head = False
                    tensor.wait_ge(sa, a_cnt[o])
                lh = None
                for c in range(NCH):
                    if head:
                        tensor.wait_ge(sv, pos + 1 + c * (SPLIT_HEAD + 1))
                    lh = xb[:, (2 * pos + c) * M : (2 * pos + c + 1) * M]
                    tensor.ldweights(lh, tile_position=(0, 0))
                    tensor.ldweights(lh, tile_position=(0, 64))
                    sp_ = last and c == NCH - 1
                    st = first and c == 0
                    off = 0
                    for j, w in enumerate(QSPLITS):
                        base = c * NSH + off
                        i1 = mm(
                            qs[j][0:M, :], j, lh,
                            moving(pos, o, base, base + w),
                            (0, 0), st, sp_,
                        )
                        i2 = mm(
                            qs[j][M : 2 * M, :], j, lh,
                            moving(pos, o, base + NA, base + NA + w),
                            (0, 64), st, sp_,
                        )
                        if sp_:
                            # region completion order: q0A, q0B, q1A, q1B
                            i1.then_inc(sp)
                            i2.then_inc(sp)
                        off += w
                if PAD_JUNK and pos <= 11:
                    # tiny junk MMs to keep the HAM activity window busy
                    # while the PE waits for the next plane (denser early,
                    # where production gaps are longest)
                    for _p in range(2 if pos <= 5 else 1):
                        pj = tensor.matmul(
                            psw[0:M, 0:M], lh, junk[:, 0:M],
                            start=True, stop=True,
                        )
                        pj.ins.ldweights = False

    return nc


def _prep(x, bp):
    x = np.asarray(x, dtype=np.float32)
    bp = np.asarray(bp)
    bytes_ = bp.astype(np.uint8)  # values are 0..255 by construction
    B = bytes_.reshape(OUT_F, IN_F // 8)
    # x[m, k] with k = PACK*(128*c + p) + o  ->  xr4[p, c, o, m]
    xr4 = np.ascontiguousarray(
        x.reshape(M, NCH, 128, PACK).transpose(2, 1, 3, 0)
    ).astype(np.float64)
    # reorder planes into consumption order; device weight for plane o is
    # w = alpha + beta*bit (act planes: w = +-(2*bit-1)), so the matching
    # x block is scaled to 2x/beta, making each product 2xb + alpha*x'.
    # corr accumulates what must be subtracted at the end:
    #   sum_k alpha*x'_bf16[m,k]  +  sum_{k not in act planes} x[m,k]
    xh = np.empty((128, NPAIR, NCH, M), ml_dtypes.bfloat16)
    corr = np.zeros(M, np.float64)
    for pos, (eng, o) in enumerate(SCHED):
        s = _shift(o)
        if eng == "act":
            # ACT planes produce exact +-1 weights via byte Sign passes
            # (+1 when the bit is set): w = 2b-1, x block = +x
            xh[:, pos] = xr4[:, :, o, :].astype(ml_dtypes.bfloat16)
            continue
        _, _, alpha, beta = _recipe(s)
        blk = (xr4[:, :, o, :] * (2.0 / beta)).astype(ml_dtypes.bfloat16)
        xh[:, pos] = blk
        if alpha != 0.0:
            # alpha * sum over this plane's x' values, per m (both chunks)
            corr += alpha * blk.astype(np.float64).sum(axis=(0, 1))
        corr += xr4[:, :, o, :].sum(axis=(0, 1))
    xh = xh.reshape(128, -1)
    rsv = (-corr).astype(np.float32)
    rs128 = np.concatenate([rsv, rsv]).reshape(128, 1)
    in_maps = []
    for cid in range(NCORES):
        Bc = np.ascontiguousarray(B[cid * NSH : (cid + 1) * NSH])  # [1376, 512] u8
        Wd = Bc.view(_np_word)  # [1376, NW] little-endian words
        bptT = np.ascontiguousarray(Wd.T)  # [NW=256, 1376]
        in_maps.append(
            {"bpt": np.ascontiguousarray(bptT), "xr": xh, "rs": rs128}
        )
    return in_maps


def _run(x, bp, trace=False):
    in_maps = _prep(x, bp)
    nc = _build()
    res = run_bass_kernel_spmd(nc, in_maps, list(range(NCORES)), trace=trace)
    outs = [np.asarray(res.results[c]["out"]) for c in range(NCORES)]
    full = np.concatenate(outs, axis=1).astype(np.float32)
    return full, res


def kernel(x, bp):
    out, _ = _run(x, bp, trace=False)
    return out
